# revision 1
# baseline (speedup 1.0000x reference)
"""Multi-head self-attention Trainium2 kernel (8 NeuronCores).

Problem: B=4, S=2048, D=1024, H=8 heads (HD=128).
  qkv = x @ qkv_w.T + qkv_b ; q,k,v = split(qkv)
  q = (q @ q_w.T + q_b)  (same k, v) -> [B,H,S,HD]
  scores = q k^T * HD^-0.5, masked softmax (attn_mask==1 -> -inf), o = attn @ v
  out = o @ out_w.T + out_b

Sharding: 8 cores = 4 batches x 2 head-groups (4 heads each).
Core c: batch b = c % 4, head-group g = c // 4.

Host-side algebraic folding: the qkv projection and per-stream q/k/v
projections are both linear, so they are composed into single effective
weights (W_eff = w @ qkv_w_slice), halving device matmul work. The
out-projection is row-parallel across head-groups; the two partial
outputs per batch are summed on host (the unshard step) with out_b.

Device flow per core (all matmuls bf16 with fp32 PSUM accumulation):
  qT_h[HD,S], kT_h[HD,S] = W x^T      (contraction over D on partitions)
  v[S, 4*HD]                          (natural layout)
  per head, per q-half (1024 q), software-pipelined 2 chunks deep:
    for kc in 16 k-chunks:
      sT = kT_h[:,kc]^T @ qT_h        [128 k, 1024 q]   (PE -> PSUM f32)
      p  = exp(SCALE * sT)            (ACT -> bf16 SBUF)
      pm = p * keepT[kc]              (DVE; keep = attn_mask.T == 0)
      oT += v[kc]^T-as-lhsT @ pm      -> oT[HD, q]      (PE, PSUM accum)
      dB += ones^T @ pm               broadcast denominator (PE, PSUM)
    oT_sb = oT * exp(-ln(dB))         softmax normalization (ACT+DVE -> bf16)
  out_partial[s,:] = sum_h oT_h[:,s_chunk]^T @ outwT_h   (+host bias/sum)
"""

import os
import sys
import types

sys.path.insert(0, "/opt/trn_rl_repo")

import numpy as np
import ml_dtypes

BF16 = ml_dtypes.bfloat16

B, S, D, H, HD = 4, 2048, 1024, 8, 128
HG = 2           # head groups
HPG = H // HG    # heads per group (4)
GD = HPG * HD    # dims per group (512)
SCALE = float(HD) ** -0.5
NKC = S // 128   # 16 k chunks
NSC = S // 128   # 16 s chunks
ND = D // 128    # 8 d chunks

_cached = {}


def _install_ntff_hook_shim():
    """The agent image's antenv lacks axon_hooks; shim it so trace works."""
    if "antenv.axon_hooks" in sys.modules:
        return
    try:
        import trn_agent_boot.trn_boot as _tb

        _hook = _tb._ntff_profile_via_ctypes("/opt/axon/libaxon_pjrt.so")
    except Exception:
        _hook = None
    _m = types.ModuleType("antenv.axon_hooks")
    _m.get_axon_ntff_profile_hook = lambda: _hook
    sys.modules["antenv.axon_hooks"] = _m


def _split_waits(nc, mybir, maxw=1):
    """Walrus in this image allows only one sync wait per instruction;
    hoist extra waits onto preceding NoOps on the same engine."""
    n_new = 0
    for fn in nc.m.functions:
        for bb in fn.blocks:
            newlist = []
            for inst in bb.instructions:
                si = inst.sync_info
                if si is not None and si.on_wait is not None and len(si.on_wait) > maxw:
                    waits = list(si.on_wait)
                    extra, keep = waits[:-maxw], waits[-maxw:]
                    while extra:
                        chunk, extra = extra[:maxw], extra[maxw:]
                        nop = mybir.InstNoOp(name=f"I-waitsplit-{nc.next_id()}")
                        nop.engine = inst.engine
                        nop.sync_info = mybir.SyncInfo(on_wait=chunk, on_update=[])
                        newlist.append(nop)
                        n_new += 1
                    si.on_wait = keep
                newlist.append(inst)
            bb.instructions = newlist
    return n_new


def _build_program(use_vbias=True):
    import concourse.bass as bass
    import concourse.mybir as mybir
    import concourse.tile as tile

    f32 = mybir.dt.float32
    bf16 = mybir.dt.bfloat16
    Exp = mybir.ActivationFunctionType.Exp
    Ident = mybir.ActivationFunctionType.Identity
    Ln = mybir.ActivationFunctionType.Ln

    nc = bass.Bass()

    # DRAM parameters (per-core shards, pre-tiled on host)
    xT = nc.declare_dram_parameter("xT", [ND, 128, S], bf16, isOutput=False)
    wqT = nc.declare_dram_parameter("wqT", [ND, 128, GD], bf16, isOutput=False)
    wkT = nc.declare_dram_parameter("wkT", [ND, 128, GD], bf16, isOutput=False)
    wvT = nc.declare_dram_parameter("wvT", [ND, 128, GD], bf16, isOutput=False)
    bq = nc.declare_dram_parameter("bq", [128, HPG], f32, isOutput=False)
    bk = nc.declare_dram_parameter("bk", [128, HPG], f32, isOutput=False)
    bvrow = nc.declare_dram_parameter("bvrow", [1, GD], bf16, isOutput=False)
    outwT = nc.declare_dram_parameter("outwT", [HPG, 128, D], bf16, isOutput=False)
    keepT = nc.declare_dram_parameter("keepT", [NKC, 128, S], bf16, isOutput=False)
    out = nc.declare_dram_parameter("out", [S, D], f32, isOutput=True)

    with tile.TileContext(nc) as tc:
        import contextlib

        with contextlib.ExitStack() as ctx:
            # --- pools ---
            # xT and keepT share one 16-slot rotation of [128, S] bf16 tiles.
            p_big = ctx.enter_context(tc.tile_pool(name="big2k", bufs=16))
            p_pers = ctx.enter_context(tc.tile_pool(name="pers", bufs=1))
            p_pm = ctx.enter_context(tc.tile_pool(name="pm", bufs=10))
            p_sm = ctx.enter_context(tc.tile_pool(name="small", bufs=2))
            pp_big = ctx.enter_context(tc.tile_pool(name="ppbig", bufs=2, space="PSUM"))
            pp_sm = ctx.enter_context(tc.tile_pool(name="ppsm", bufs=4, space="PSUM"))

            # --- constants + small inputs ---
            ones128 = p_pers.tile([128, 128], bf16, tag="ones128", name="ones128")
            nc.vector.memset(ones128, 1.0)

            # --- batched loads: few large DMAs (each dma_start costs ~600ns
            # of sequencer issue time); q weights first, then xT in 4 chunks
            # so the q projection groups start after ~1 chunk ---
            w_sb = {}
            xt_tiles = []
            for d in range(ND):
                t = p_pers.tile([128, GD], bf16, tag=f"wq{d}", name=f"wq{d}")
                nc.sync.dma_start(out=t, in_=wqT[d])
                w_sb[("q", d)] = t
                t = p_big.tile([128, S], bf16, tag="big2k", name="big2k")
                nc.sync.dma_start(out=t, in_=xT[d])
                xt_tiles.append(t)
            for name, drm in (("k", wkT), ("v", wvT)):
                for d in range(ND):
                    t = p_pers.tile([128, GD], bf16, tag=f"w{name}{d}", name=f"w{name}{d}")
                    nc.sync.dma_start(out=t, in_=drm[d])
                    w_sb[(name, d)] = t

            bq_sb = p_pers.tile([128, HPG], f32, tag="bq", name="bq_sb")
            nc.sync.dma_start(out=bq_sb, in_=bq[:, :])
            bk_sb = p_pers.tile([128, HPG], f32, tag="bk", name="bk_sb")
            nc.sync.dma_start(out=bk_sb, in_=bk[:, :])
            bv_sb = None
            if use_vbias:
                bv_sb = p_pers.tile([1, GD], bf16, tag="bv", name="bv_sb")
                nc.sync.dma_start(out=bv_sb, in_=bvrow[:, :])

            def w_sl(name, d):
                return w_sb[(name, d)]

            def xT_sl(d, lo, hi):
                return xt_tiles[d][:, lo:hi]

            keep_tiles = [None] * NKC
            for kc in range(8):
                t = p_big.tile([128, S], bf16, tag="big2k", name="big2k")
                nc.sync.dma_start(out=t, in_=keepT[kc])
                keep_tiles[kc] = t

            def keep_sl(kc, lo, hi):
                return keep_tiles[kc][:, lo:hi]

            outw_sb = []
            for h in range(HPG):
                t = p_pers.tile([128, D], bf16, tag=f"outw{h}", name=f"outw{h}")
                nc.sync.dma_start(out=t, in_=outwT[h])
                outw_sb.append(t)

            # --- projections (d-major, 4 concurrent PSUM accumulators so the
            # d=0 matmuls of a group start as soon as xT[0]/w[0] land) ---
            qT_sb = [p_pers.tile([128, S], bf16, tag=f"qT{h}", name=f"qT{h}") for h in range(HPG)]
            kT_sb = [p_pers.tile([128, S], bf16, tag=f"kT{h}", name=f"kT{h}") for h in range(HPG)]

            units = []  # (stream, head, quarter)
            for name, dst, bias in (("q", qT_sb, bq_sb), ("k", kT_sb, bk_sb)):
                for h in range(HPG):
                    for qu in range(4):
                        units.append((name, dst, bias, h, qu))
            for gstart in range(0, len(units), 4):
                group = units[gstart:gstart + 4]
                pss = [
                    pp_sm.tile([128, 512], f32, tag="ppsm", name="ppsm")
                    for _ in group
                ]
                for d in range(ND):
                    for (name, dst, bias, h, qu), ps in zip(group, pss):
                        nc.tensor.matmul(
                            ps,
                            lhsT=w_sl(name, d)[:, h * 128:(h + 1) * 128],
                            rhs=xT_sl(d, qu * 512, (qu + 1) * 512),
                            start=(d == 0),
                            stop=(d == ND - 1),
                        )
                for (name, dst, bias, h, qu), ps in zip(group, pss):
                    nc.scalar.activation(
                        out=dst[h][:, qu * 512:(qu + 1) * 512],
                        in_=ps,
                        func=Ident,
                        bias=bias[:, h:h + 1],
                    )

            v_sb = [p_pers.tile([128, GD], bf16, tag=f"v{sc}", name=f"v{sc}") for sc in range(NSC)]
            for sc in range(NSC):
                ps = pp_sm.tile([128, GD], f32, tag="ppsm", name="ppsm")
                for d in range(ND):
                    nc.tensor.matmul(
                        ps,
                        lhsT=xT_sl(d, sc * 128, (sc + 1) * 128),
                        rhs=w_sl("v", d),
                        start=(d == 0),
                        stop=(d == ND - 1) and not use_vbias,
                    )
                if use_vbias:
                    # bias via K=1 ones row
                    nc.tensor.matmul(
                        ps,
                        lhsT=ones128[0:1, :],
                        rhs=bv_sb,
                        start=False,
                        stop=True,
                    )
                nc.vector.tensor_copy(v_sb[sc], ps)

            # --- second half of keepT (reuses xT slots once proj done) ---
            for kc in range(8, NKC):
                t = p_big.tile([128, S], bf16, tag="big2k", name="big2k")
                nc.sync.dma_start(out=t, in_=keepT[kc])
                keep_tiles[kc] = t

            # --- attention ---
            oT_sb = [p_pers.tile([128, S], bf16, tag=f"oT{h}", name=f"oT{h}") for h in range(HPG)]
            for h in range(HPG):
                for half in range(2):
                    q0 = half * 1024
                    o_ps = [pp_sm.tile([128, 512], f32, tag="ppsm", name="ppsm") for _ in range(2)]
                    d_ps = [pp_sm.tile([128, 512], f32, tag="ppsm", name="ppsm") for _ in range(2)]

                    def consume(kc, pm):
                        # oT/dB accumulation for chunk kc, issued two stages
                        # late so the PE never waits on ACT/DVE for this kc
                        for qq in range(2):
                            nc.tensor.matmul(
                                o_ps[qq],
                                lhsT=v_sb[kc][:, h * 128:(h + 1) * 128],
                                rhs=pm[:, qq * 512:(qq + 1) * 512],
                                start=(kc == 0),
                                stop=(kc == NKC - 1),
                            )
                        for qq in range(2):
                            nc.tensor.matmul(
                                d_ps[qq],
                                lhsT=ones128,
                                rhs=pm[:, qq * 512:(qq + 1) * 512],
                                start=(kc == 0),
                                stop=(kc == NKC - 1),
                            )

                    pending = []  # [(kc, pm)] — 2-stage delay
                    for kc in range(NKC):
                        sT = pp_big.tile([128, 1024], f32, tag="ppbig", name="ppbig")
                        for nn in range(2):
                            nc.tensor.matmul(
                                sT[:, nn * 512:(nn + 1) * 512],
                                lhsT=kT_sb[h][:, kc * 128:(kc + 1) * 128],
                                rhs=qT_sb[h][:, q0 + nn * 512:q0 + (nn + 1) * 512],
                                start=True,
                                stop=True,
                            )
                        p = p_pm.tile([128, 1024], bf16, tag="pm", name="pm")
                        nc.scalar.activation(out=p, in_=sT, func=Exp, scale=SCALE)
                        pm = p_pm.tile([128, 1024], bf16, tag="pm", name="pm")
                        nc.vector.tensor_mul(
                            pm, p, keep_sl(kc, q0, q0 + 1024)
                        )
                        pending.append((kc, pm))
                        if len(pending) > 2:
                            consume(*pending.pop(0))
                    for item in pending:
                        consume(*item)
                    for qq in range(2):
                        # 1/d via exp(-ln(d)) on ACT: frees the PSUM
                        # accumulators fast and keeps DVE reciprocal (which
                        # measures ~6 cyc/elem) off the critical path.
                        lnd = p_sm.tile([128, 512], f32, tag="lnd", name="lnd")
                        nc.scalar.activation(out=lnd, in_=d_ps[qq], func=Ln)
                        rdb = p_sm.tile([128, 512], f32, tag="rdb", name="rdb")
                        nc.scalar.activation(out=rdb, in_=lnd, func=Exp, scale=-1.0)
                        nc.vector.tensor_mul(
                            oT_sb[h][:, q0 + qq * 512:q0 + (qq + 1) * 512],
                            o_ps[qq],
                            rdb,
                        )

            # --- output projection (partial; host adds the two groups + bias) ---
            for sc in range(NSC):
                ps = pp_big.tile([128, 1024], f32, tag="ppbig", name="ppbig")
                for h in range(HPG):
                    for nn in range(2):
                        nc.tensor.matmul(
                            ps[:, nn * 512:(nn + 1) * 512],
                            lhsT=oT_sb[h][:, sc * 128:(sc + 1) * 128],
                            rhs=outw_sb[h][:, nn * 512:(nn + 1) * 512],
                            start=(h == 0),
                            stop=(h == HPG - 1),
                        )
                osb = p_sm.tile([128, 1024], f32, tag="osb", name="osb")
                nc.vector.tensor_copy(osb, ps)
                nc.sync.dma_start(out=out[sc * 128:(sc + 1) * 128, :], in_=osb)

    _split_waits(nc, mybir, maxw=1)
    return nc


def _prep_core_inputs(x, attn_mask, qkv_w, qkv_b, q_w, q_b, k_w, k_b, v_w, v_b,
                      out_w):
    """Host-side: fold projections, shard, pre-transpose/tile, cast."""
    f = np.float32
    x = np.asarray(x, f)
    qkv_w = np.asarray(qkv_w, f)
    qkv_b = np.asarray(qkv_b, f)
    Ws = {}
    bs = {}
    for i, (w, b) in enumerate(((q_w, q_b), (k_w, k_b), (v_w, v_b))):
        w = np.asarray(w, f)
        b = np.asarray(b, f)
        sl = slice(i * D, (i + 1) * D)
        Ws[i] = w @ qkv_w[sl]              # [D, D] effective
        bs[i] = b + w @ qkv_b[sl]          # [D]
    out_wT = np.ascontiguousarray(np.asarray(out_w, f).T)  # [D(hd), D(model)]

    keepT = (np.asarray(attn_mask).T == 0).astype(BF16)    # [k, q]
    keepT_t = np.ascontiguousarray(keepT).reshape(NKC, 128, S)

    xT_all = []
    for b_i in range(B):
        xb = np.ascontiguousarray(x[b_i].T.astype(BF16))   # [D, S]
        xT_all.append(xb.reshape(ND, 128, S))

    maps = []
    for c in range(8):
        b_i = c % B
        g = c // B
        sl = slice(g * GD, (g + 1) * GD)
        m = {
            "xT": xT_all[b_i],
            "wqT": np.ascontiguousarray(Ws[0][sl].T.astype(BF16)).reshape(ND, 128, GD),
            "wkT": np.ascontiguousarray(Ws[1][sl].T.astype(BF16)).reshape(ND, 128, GD),
            "wvT": np.ascontiguousarray(Ws[2][sl].T.astype(BF16)).reshape(ND, 128, GD),
            "bq": np.ascontiguousarray(bs[0][sl].reshape(HPG, 128).T.astype(f)),
            "bk": np.ascontiguousarray(bs[1][sl].reshape(HPG, 128).T.astype(f)),
            "bvrow": bs[2][sl].astype(BF16).reshape(1, GD),
            "outwT": np.ascontiguousarray(out_wT[sl].astype(BF16)).reshape(HPG, 128, D),
            "keepT": keepT_t,
        }
        maps.append(m)
    return maps


def kernel(x, attn_mask, qkv_w, qkv_b, q_w, q_b, k_w, k_b, v_w, v_b,
           out_w, out_b, _trace=False):
    _install_ntff_hook_shim()
    from concourse.bass_utils import run_bass_kernel_spmd

    in_maps = _prep_core_inputs(
        x, attn_mask, qkv_w, qkv_b, q_w, q_b, k_w, k_b, v_w, v_b, out_w
    )
    use_vbias = bool(np.any(np.asarray(in_maps[0]["bvrow"], np.float32) != 0))
    key = ("nc", use_vbias)
    if key not in _cached:
        _cached[key] = _build_program(use_vbias=use_vbias)
    nc = _cached[key]
    core_ids = list(range(8))
    try:
        res = run_bass_kernel_spmd(nc, in_maps, core_ids, trace=_trace)
    except Exception:
        # transient NRT device wedge recovers on retry
        res = run_bass_kernel_spmd(nc, in_maps, core_ids, trace=_trace)
    _cached["last_result"] = res

    out_b = np.asarray(out_b, np.float32)
    full = np.empty((B, S, D), np.float32)
    for b_i in range(B):
        full[b_i] = (
            res.results[b_i]["out"] + res.results[b_i + B]["out"] + out_b
        )
    return full

